# revision 2
# baseline (speedup 1.0000x reference)
"""Distributed cross-attention energy kernel for one TRN2 chip (8 NeuronCores).

Computes  -sum_i logsumexp_j( (xq @ Wq) @ (xk @ Wk)^T )[i, j]  for
Nq = Nk = 8192, D = 256 (fp32 inputs).

Strategy
--------
Algebra: scores = xq @ (Wq @ Wk^T) @ xk^T.  The host folds the two weight
matrices into A = Wq @ Wk^T (256x256, trivially cheap) so each core runs
two chained matmuls: QaT = A^T @ xqT (local rows), then score row-blocks
Qa @ xk^T, consumed tile-by-tile by an online logsumexp:

  per 128-row q-block, per 2048-wide j-tile:
      PE   : 8x  [128x128] @ [128x512] bf16 matmuls -> PSUM fp32
      DVE  : reduce_max (negated) over the PSUM tile -> -m_t
      ACT  : exp(s - m_t) with fused per-partition accumulate -> S_t

Each core ships per-tile (-m_t, S_t) pairs ([128, 8, 4] fp32 each) back to
the host, which merges them in float64:  lse = M + log(sum_t S_t e^{m_t-M}).
logsumexp compresses relative error, so bf16 matmul inputs are safe: tile
sums carry ~0.2% relative noise, which perturbs each row's lse by only
~log(1.002) ~ 2e-3 absolute against |lse| ~ 1.7e4.

Sharding: xq rows split across 8 cores (1024 rows each); A and xk^T are
replicated.  All inputs are pre-transposed + bf16-cast on the host so the
contraction dim (D=256, as 2 chunks of 128) lands on SBUF partitions.
"""

import sys

_TRN_REPO = "/opt/trn_rl_repo"
if _TRN_REPO not in sys.path:
    sys.path.insert(0, _TRN_REPO)

import ml_dtypes
import numpy as np

N_CORES = 8
NQ, NK, D = 8192, 8192, 256
ROWS = NQ // N_CORES        # q-rows per core
QB = ROWS // 128            # q-blocks (of 128 rows) per core
JT = 4                      # j-tiles per q-block row sweep
JW = NK // JT               # j-tile width (2048 fp32 = 4 PSUM banks)
BF16 = ml_dtypes.bfloat16

_COMPILED = None


def _build_nc():
    import concourse.bacc as bacc
    import concourse.mybir as mybir
    import concourse.tile as tile

    nc = bacc.Bacc("TRN2", target_bir_lowering=False, debug=False)
    f32 = mybir.dt.float32
    bf16 = mybir.dt.bfloat16
    EXP = mybir.ActivationFunctionType.Exp
    AX = mybir.AxisListType.X

    amat_d = nc.dram_tensor("amat", [2, 128, D], bf16, kind="ExternalInput").ap()
    xqt_d = nc.dram_tensor("xqt", [2, 128, ROWS], bf16, kind="ExternalInput").ap()
    xkt_d = nc.dram_tensor("xkt", [2, 128, NK], bf16, kind="ExternalInput").ap()
    negm_d = nc.dram_tensor("negm", [128, QB, JT], f32, kind="ExternalOutput").ap()
    ssum_d = nc.dram_tensor("ssum", [128, QB, JT], f32, kind="ExternalOutput").ap()

    with tile.TileContext(nc) as tc:
        with (
            tc.tile_pool(name="big", bufs=1) as big,
            tc.tile_pool(name="psum", bufs=2, space="PSUM") as psum,
        ):
            negm = big.tile([128, QB, JT], f32, name="negm", tag="negm")
            ssum = big.tile([128, QB, JT], f32, name="ssum", tag="ssum")
            amat = [big.tile([128, D], bf16, name=f"amat{e}", tag=f"amat{e}") for e in range(2)]
            xqt = [big.tile([128, ROWS], bf16, name=f"xqt{e}", tag=f"xqt{e}") for e in range(2)]
            xkt = [
                [big.tile([128, JW], bf16, name=f"xkt{e}_{j}", tag=f"xkt{e}_{j}") for j in range(JT)]
                for e in range(2)
            ]
            qat = [big.tile([128, ROWS], bf16, name=f"qat{d}", tag=f"qat{d}") for d in range(2)]

            for e in range(2):
                nc.sync.dma_start(amat[e][:], amat_d[e])
                nc.sync.dma_start(xqt[e][:], xqt_d[e])
            for e in range(2):
                for j in range(JT):
                    nc.sync.dma_start(xkt[e][j][:], xkt_d[e][:, j * JW : (j + 1) * JW])

            # QaT[d-chunk] = sum_e (A[e-chunk, d-cols])^T @ xqT[e-chunk]
            for d in range(2):
                psq = psum.tile([128, ROWS], f32, name="psq", tag="ps")
                for n in range(ROWS // 512):
                    for e in range(2):
                        nc.tensor.matmul(
                            psq[:, n * 512 : (n + 1) * 512],
                            amat[e][:, d * 128 : (d + 1) * 128],
                            xqt[e][:, n * 512 : (n + 1) * 512],
                            start=(e == 0),
                            stop=(e == 1),
                        )
                nc.vector.tensor_copy(qat[d][:], psq[:])

            # score tiles + online logsumexp partials
            for b in range(QB):
                for j in range(JT):
                    ps = psum.tile([128, JW], f32, name="ps", tag="ps")
                    for s in range(JW // 512):
                        for d in range(2):
                            nc.tensor.matmul(
                                ps[:, s * 512 : (s + 1) * 512],
                                qat[d][:, b * 128 : (b + 1) * 128],
                                xkt[d][j][:, s * 512 : (s + 1) * 512],
                                start=(d == 0),
                                stop=(d == 1),
                            )
                    nc.vector.reduce_max(
                        negm[:, b, j : j + 1], ps[:], axis=AX, negate=True
                    )
                    nc.scalar.activation(
                        ps[:],
                        ps[:],
                        EXP,
                        bias=negm[:, b, j : j + 1],
                        scale=1.0,
                        accum_out=ssum[:, b, j : j + 1],
                    )

            nc.sync.dma_start(negm_d[:], negm[:])
            nc.sync.dma_start(ssum_d[:], ssum[:])

    nc.compile()
    return nc


def _get_compiled():
    global _COMPILED
    if _COMPILED is None:
        _COMPILED = _build_nc()
    return _COMPILED


def make_in_maps(xq, xk, Wq, Wk):
    """Host-side shard + layout prep: transpose, bf16-cast, fold A = Wq Wk^T."""
    A = (Wq.astype(np.float32) @ Wk.astype(np.float32).T).astype(BF16)
    amat = np.ascontiguousarray(A.reshape(2, 128, D))
    xkt = np.ascontiguousarray(xk.T.astype(BF16).reshape(2, 128, NK))
    in_maps = []
    for c in range(N_CORES):
        xq_c = xq[c * ROWS : (c + 1) * ROWS]
        xqt = np.ascontiguousarray(xq_c.T.astype(BF16).reshape(2, 128, ROWS))
        in_maps.append({"amat": amat, "xqt": xqt, "xkt": xkt})
    return in_maps


def combine_outputs(results):
    """Merge per-core (-m_t, S_t) tile stats into the final scalar (float64)."""
    total = 0.0
    for res in results:
        m = -res["negm"].astype(np.float64)  # [128, QB, JT] tile maxima
        s = res["ssum"].astype(np.float64)  # [128, QB, JT] tile exp-sums
        mrow = m.max(axis=-1)
        t = (s * np.exp(m - mrow[..., None])).sum(axis=-1)
        lse = mrow + np.log(t)
        total += lse.sum()
    return np.float32(-total)


def kernel(xq, xk, Wq, Wk):
    from concourse.bass_utils import run_bass_kernel_spmd

    xq = np.asarray(xq, dtype=np.float32)
    xk = np.asarray(xk, dtype=np.float32)
    Wq = np.asarray(Wq, dtype=np.float32)
    Wk = np.asarray(Wk, dtype=np.float32)

    nc = _get_compiled()
    in_maps = make_in_maps(xq, xk, Wq, Wk)
    out = run_bass_kernel_spmd(nc, in_maps, list(range(N_CORES)))
    return combine_outputs(out.results)


# revision 4
# speedup vs baseline: 1.0254x; 1.0254x over previous
"""Distributed cross-attention energy kernel for one TRN2 chip (8 NeuronCores).

Computes  -sum_i logsumexp_j( (xq @ Wq) @ (xk @ Wk)^T )[i, j]  for
Nq = Nk = 8192, D = 256 (fp32 inputs).

Strategy
--------
Algebra: scores = xq @ (Wq @ Wk^T) @ xk^T.  The host folds the two weight
matrices into A = Wq @ Wk^T (256x256, trivially cheap) so each core runs
two chained matmuls: QaT = A^T @ xqT (local rows), then score row-blocks
Qa @ xk^T, consumed tile-by-tile by an online logsumexp:

  per 128-row q-block, per 2048-wide j-tile:
      PE   : 8x  [128x128] @ [128x512] bf16 matmuls -> PSUM fp32
      DVE  : reduce_max (negated) over the PSUM tile -> -m_t
      ACT  : exp(s - m_t) with fused per-partition accumulate -> S_t

Each core ships per-tile (-m_t, S_t) pairs ([128, 8, 4] fp32 each) back to
the host, which merges them in float64:  lse = M + log(sum_t S_t e^{m_t-M}).
logsumexp compresses relative error, so bf16 matmul inputs are safe: tile
sums carry ~0.2% relative noise, which perturbs each row's lse by only
~log(1.002) ~ 2e-3 absolute against |lse| ~ 1.7e4.

Sharding: xq rows split across 8 cores (1024 rows each); A and xk^T are
replicated.  All inputs are pre-transposed + bf16-cast on the host so the
contraction dim (D=256, as 2 chunks of 128) lands on SBUF partitions.
"""

import sys

_TRN_REPO = "/opt/trn_rl_repo"
if _TRN_REPO not in sys.path:
    sys.path.insert(0, _TRN_REPO)

import ml_dtypes
import numpy as np

N_CORES = 8
NQ, NK, D = 8192, 8192, 256
ROWS = NQ // N_CORES        # q-rows per core
QB = ROWS // 128            # q-blocks (of 128 rows) per core
JT = 4                      # j-tiles per q-block row sweep
JW = NK // JT               # j-tile width (2048 fp32 = 4 PSUM banks)
BF16 = ml_dtypes.bfloat16

_COMPILED = None


def _build_nc():
    import concourse.bacc as bacc
    import concourse.mybir as mybir
    import concourse.tile as tile

    nc = bacc.Bacc("TRN2", target_bir_lowering=False, debug=False)
    f32 = mybir.dt.float32
    bf16 = mybir.dt.bfloat16
    EXP = mybir.ActivationFunctionType.Exp
    AX = mybir.AxisListType.X

    amat_d = nc.dram_tensor("amat", [2, 128, D], bf16, kind="ExternalInput").ap()
    xqt_d = nc.dram_tensor("xqt", [2, 128, ROWS], bf16, kind="ExternalInput").ap()
    xkt_d = nc.dram_tensor("xkt", [2, 128, NK], bf16, kind="ExternalInput").ap()
    negm_d = nc.dram_tensor("negm", [128, QB, JT], f32, kind="ExternalOutput").ap()
    ssum_d = nc.dram_tensor("ssum", [128, QB, JT], f32, kind="ExternalOutput").ap()

    with tile.TileContext(nc) as tc:
        with (
            tc.tile_pool(name="big", bufs=1) as big,
            tc.tile_pool(name="psum", bufs=2, space="PSUM") as psum,
        ):
            negm = big.tile([128, QB, JT], f32, name="negm", tag="negm")
            ssum = big.tile([128, QB, JT], f32, name="ssum", tag="ssum")
            amat = [big.tile([128, D], bf16, name=f"amat{e}", tag=f"amat{e}") for e in range(2)]
            xqt = [big.tile([128, ROWS], bf16, name=f"xqt{e}", tag=f"xqt{e}") for e in range(2)]
            xkt = [
                [big.tile([128, JW], bf16, name=f"xkt{e}_{j}", tag=f"xkt{e}_{j}") for j in range(JT)]
                for e in range(2)
            ]
            qat = [big.tile([128, ROWS], bf16, name=f"qat{d}", tag=f"qat{d}") for d in range(2)]

            # PE warm-up: dense dummy matmuls at t=0 so the HAM clock-gate
            # reaches 8/8 (2.4 GHz) before real matmuls start; otherwise the
            # whole kernel runs at the 1.2 GHz cold rate (measured 407ns/MM).
            scratch = big.tile([128, 512], bf16, name="warm_sb", tag="warm_sb")
            nc.vector.memset(scratch[:], 0.0)
            warm_ps = psum.tile([128, JW], f32, name="warm_ps", tag="ps")
            for w in range(16):
                nc.tensor.matmul(
                    warm_ps[:, :512],
                    scratch[:, :128],
                    scratch[:],
                    start=True,
                    stop=True,
                )

            for e in range(2):
                nc.sync.dma_start(amat[e][:], amat_d[e])
                nc.sync.dma_start(xqt[e][:], xqt_d[e])
            for j in range(JT):
                for e in range(2):
                    nc.sync.dma_start(xkt[e][j][:], xkt_d[e][:, j * JW : (j + 1) * JW])

            # QaT[d-chunk] = sum_e (A[e-chunk, d-cols])^T @ xqT[e-chunk]
            for d in range(2):
                psq = psum.tile([128, ROWS], f32, name="psq", tag="ps")
                for n in range(ROWS // 512):
                    for e in range(2):
                        nc.tensor.matmul(
                            psq[:, n * 512 : (n + 1) * 512],
                            amat[e][:, d * 128 : (d + 1) * 128],
                            xqt[e][:, n * 512 : (n + 1) * 512],
                            start=(e == 0),
                            stop=(e == 1),
                        )
                nc.scalar.copy(qat[d][:], psq[:])

            # score tiles + online logsumexp partials
            for b in range(QB):
                for j in range(JT):
                    ps = psum.tile([128, JW], f32, name="ps", tag="ps")
                    for s in range(JW // 512):
                        for d in range(2):
                            nc.tensor.matmul(
                                ps[:, s * 512 : (s + 1) * 512],
                                qat[d][:, b * 128 : (b + 1) * 128],
                                xkt[d][j][:, s * 512 : (s + 1) * 512],
                                start=(d == 0),
                                stop=(d == 1),
                            )
                    nc.vector.reduce_max(
                        negm[:, b, j : j + 1], ps[:], axis=AX, negate=True
                    )
                    nc.scalar.activation(
                        ps[:],
                        ps[:],
                        EXP,
                        bias=negm[:, b, j : j + 1],
                        scale=1.0,
                        accum_out=ssum[:, b, j : j + 1],
                    )

            nc.sync.dma_start(negm_d[:], negm[:])
            nc.sync.dma_start(ssum_d[:], ssum[:])

    nc.compile()
    return nc


def _get_compiled():
    global _COMPILED
    if _COMPILED is None:
        _COMPILED = _build_nc()
    return _COMPILED


def make_in_maps(xq, xk, Wq, Wk):
    """Host-side shard + layout prep: transpose, bf16-cast, fold A = Wq Wk^T."""
    A = (Wq.astype(np.float32) @ Wk.astype(np.float32).T).astype(BF16)
    amat = np.ascontiguousarray(A.reshape(2, 128, D))
    xkt = np.ascontiguousarray(xk.T.astype(BF16).reshape(2, 128, NK))
    in_maps = []
    for c in range(N_CORES):
        xq_c = xq[c * ROWS : (c + 1) * ROWS]
        xqt = np.ascontiguousarray(xq_c.T.astype(BF16).reshape(2, 128, ROWS))
        in_maps.append({"amat": amat, "xqt": xqt, "xkt": xkt})
    return in_maps


def combine_outputs(results):
    """Merge per-core (-m_t, S_t) tile stats into the final scalar (float64)."""
    total = 0.0
    for res in results:
        m = -res["negm"].astype(np.float64)  # [128, QB, JT] tile maxima
        s = res["ssum"].astype(np.float64)  # [128, QB, JT] tile exp-sums
        mrow = m.max(axis=-1)
        t = (s * np.exp(m - mrow[..., None])).sum(axis=-1)
        lse = mrow + np.log(t)
        total += lse.sum()
    return np.float32(-total)


def kernel(xq, xk, Wq, Wk):
    from concourse.bass_utils import run_bass_kernel_spmd

    xq = np.asarray(xq, dtype=np.float32)
    xk = np.asarray(xk, dtype=np.float32)
    Wq = np.asarray(Wq, dtype=np.float32)
    Wk = np.asarray(Wk, dtype=np.float32)

    nc = _get_compiled()
    in_maps = make_in_maps(xq, xk, Wq, Wk)
    out = run_bass_kernel_spmd(nc, in_maps, list(range(N_CORES)))
    return combine_outputs(out.results)


# revision 6
# speedup vs baseline: 1.0375x; 1.0118x over previous
"""Distributed cross-attention energy kernel for one TRN2 chip (8 NeuronCores).

Computes  -sum_i logsumexp_j( (xq @ Wq) @ (xk @ Wk)^T )[i, j]  for
Nq = Nk = 8192, D = 256 (fp32 inputs).

Strategy
--------
Algebra: scores = xq @ (Wq @ Wk^T) @ xk^T.  The host folds the two weight
matrices into A = Wq @ Wk^T (256x256, trivially cheap) so each core runs
two chained matmuls: QaT = A^T @ xqT (local rows), then score row-blocks
Qa @ xk^T, consumed tile-by-tile by an online logsumexp:

  per 128-row q-block, per 2048-wide j-tile:
      PE   : 8x  [128x128] @ [128x512] bf16 matmuls -> PSUM fp32
      DVE  : reduce_max (negated) over the PSUM tile -> -m_t
      ACT  : exp(s - m_t) with fused per-partition accumulate -> S_t

Each core ships per-tile (-m_t, S_t) pairs ([128, 8, 4] fp32 each) back to
the host, which merges them in float64:  lse = M + log(sum_t S_t e^{m_t-M}).
logsumexp compresses relative error, so bf16 matmul inputs are safe: tile
sums carry ~0.2% relative noise, which perturbs each row's lse by only
~log(1.002) ~ 2e-3 absolute against |lse| ~ 1.7e4.

Sharding: xq rows split across 8 cores (1024 rows each); A and xk^T are
replicated.  All inputs are pre-transposed + bf16-cast on the host so the
contraction dim (D=256, as 2 chunks of 128) lands on SBUF partitions.
"""

import sys

_TRN_REPO = "/opt/trn_rl_repo"
if _TRN_REPO not in sys.path:
    sys.path.insert(0, _TRN_REPO)

import ml_dtypes
import numpy as np

N_CORES = 8
NQ, NK, D = 8192, 8192, 256
ROWS = NQ // N_CORES        # q-rows per core
QB = ROWS // 128            # q-blocks (of 128 rows) per core
JT = 4                      # j-tiles per q-block row sweep
JW = NK // JT               # j-tile width (2048 fp32 = 4 PSUM banks)
BF16 = ml_dtypes.bfloat16

_COMPILED = None


def _build_nc():
    import concourse.bacc as bacc
    import concourse.mybir as mybir
    import concourse.tile as tile

    nc = bacc.Bacc("TRN2", target_bir_lowering=False, debug=False)
    f32 = mybir.dt.float32
    bf16 = mybir.dt.bfloat16
    EXP = mybir.ActivationFunctionType.Exp
    AX = mybir.AxisListType.X

    amat_d = nc.dram_tensor("amat", [2, 128, D], bf16, kind="ExternalInput").ap()
    xqt_d = nc.dram_tensor("xqt", [2, 128, ROWS], bf16, kind="ExternalInput").ap()
    xkt_d = nc.dram_tensor("xkt", [2, 128, NK], bf16, kind="ExternalInput").ap()
    negm_d = nc.dram_tensor("negm", [128, QB, JT], f32, kind="ExternalOutput").ap()
    ssum_d = nc.dram_tensor("ssum", [128, QB, JT], f32, kind="ExternalOutput").ap()

    with tile.TileContext(nc) as tc:
        with (
            tc.tile_pool(name="big", bufs=1) as big,
            tc.tile_pool(name="psum", bufs=2, space="PSUM") as psum,
        ):
            negm = big.tile([128, QB, JT], f32, name="negm", tag="negm")
            ssum = big.tile([128, QB, JT], f32, name="ssum", tag="ssum")
            amat = [big.tile([128, D], bf16, name=f"amat{e}", tag=f"amat{e}") for e in range(2)]
            xqt = [big.tile([128, ROWS], bf16, name=f"xqt{e}", tag=f"xqt{e}") for e in range(2)]
            xkt = [
                [big.tile([128, JW], bf16, name=f"xkt{e}_{j}", tag=f"xkt{e}_{j}") for j in range(JT)]
                for e in range(2)
            ]
            qat = [big.tile([128, ROWS], bf16, name=f"qat{d}", tag=f"qat{d}") for d in range(2)]

            # PE warm-up: dense dummy matmuls at t=0 so the HAM clock-gate
            # reaches 8/8 (2.4 GHz) before real matmuls start; otherwise the
            # whole kernel runs at the 1.2 GHz cold rate (measured 407ns/MM).
            scratch = big.tile([128, 512], bf16, name="warm_sb", tag="warm_sb")
            nc.gpsimd.memset(scratch[:], 0.0)
            warm_ps = psum.tile([128, JW], f32, name="warm_ps", tag="ps")
            for w in range(16):
                nc.tensor.matmul(
                    warm_ps[:, :512],
                    scratch[:, :128],
                    scratch[:],
                    start=True,
                    stop=True,
                )

            for e in range(2):
                nc.sync.dma_start(amat[e][:], amat_d[e])
                nc.sync.dma_start(xqt[e][:], xqt_d[e])
            for j in range(JT):
                for e in range(2):
                    nc.sync.dma_start(xkt[e][j][:], xkt_d[e][:, j * JW : (j + 1) * JW])

            # QaT[d-chunk] = sum_e (A[e-chunk, d-cols])^T @ xqT[e-chunk].
            # n-outer / per-(n,d) psum so the first half of qat (cols 0:512,
            # feeding q-blocks 0-3) is ready after two copies.
            for n in range(ROWS // 512):
                for d in range(2):
                    psq = psum.tile([128, 512], f32, name="psq", tag="ps")
                    for e in range(2):
                        nc.tensor.matmul(
                            psq[:],
                            amat[e][:, d * 128 : (d + 1) * 128],
                            xqt[e][:, n * 512 : (n + 1) * 512],
                            start=(e == 0),
                            stop=(e == 1),
                        )
                    nc.scalar.copy(qat[d][:, n * 512 : (n + 1) * 512], psq[:])

            # score tiles + online logsumexp partials.  d-outer so the
            # stationary operand is swapped twice per tile instead of per
            # matmul (per-MM LDWEIGHTS measured 418ns/MM vs 216 streaming).
            for b in range(QB):
                for j in range(JT):
                    ps = psum.tile([128, JW], f32, name="ps", tag="ps")
                    for d in range(2):
                        for s in range(JW // 512):
                            nc.tensor.matmul(
                                ps[:, s * 512 : (s + 1) * 512],
                                qat[d][:, b * 128 : (b + 1) * 128],
                                xkt[d][j][:, s * 512 : (s + 1) * 512],
                                start=(d == 0),
                                stop=(d == 1),
                            )
                    nc.vector.reduce_max(
                        negm[:, b, j : j + 1], ps[:], axis=AX, negate=True
                    )
                    nc.scalar.activation(
                        ps[:],
                        ps[:],
                        EXP,
                        bias=negm[:, b, j : j + 1],
                        scale=1.0,
                        accum_out=ssum[:, b, j : j + 1],
                    )

            nc.sync.dma_start(negm_d[:], negm[:])
            nc.sync.dma_start(ssum_d[:], ssum[:])

    nc.compile()
    return nc


def _get_compiled():
    global _COMPILED
    if _COMPILED is None:
        _COMPILED = _build_nc()
    return _COMPILED


def make_in_maps(xq, xk, Wq, Wk):
    """Host-side shard + layout prep: transpose, bf16-cast, fold A = Wq Wk^T."""
    A = (Wq.astype(np.float32) @ Wk.astype(np.float32).T).astype(BF16)
    amat = np.ascontiguousarray(A.reshape(2, 128, D))
    xkt = np.ascontiguousarray(xk.T.astype(BF16).reshape(2, 128, NK))
    in_maps = []
    for c in range(N_CORES):
        xq_c = xq[c * ROWS : (c + 1) * ROWS]
        xqt = np.ascontiguousarray(xq_c.T.astype(BF16).reshape(2, 128, ROWS))
        in_maps.append({"amat": amat, "xqt": xqt, "xkt": xkt})
    return in_maps


def combine_outputs(results):
    """Merge per-core (-m_t, S_t) tile stats into the final scalar (float64)."""
    total = 0.0
    for res in results:
        m = -res["negm"].astype(np.float64)  # [128, QB, JT] tile maxima
        s = res["ssum"].astype(np.float64)  # [128, QB, JT] tile exp-sums
        mrow = m.max(axis=-1)
        t = (s * np.exp(m - mrow[..., None])).sum(axis=-1)
        lse = mrow + np.log(t)
        total += lse.sum()
    return np.float32(-total)


def kernel(xq, xk, Wq, Wk):
    from concourse.bass_utils import run_bass_kernel_spmd

    xq = np.asarray(xq, dtype=np.float32)
    xk = np.asarray(xk, dtype=np.float32)
    Wq = np.asarray(Wq, dtype=np.float32)
    Wk = np.asarray(Wk, dtype=np.float32)

    nc = _get_compiled()
    in_maps = make_in_maps(xq, xk, Wq, Wk)
    out = run_bass_kernel_spmd(nc, in_maps, list(range(N_CORES)))
    return combine_outputs(out.results)


# revision 8
# speedup vs baseline: 1.3478x; 1.2991x over previous
"""Distributed cross-attention energy kernel for one TRN2 chip (8 NeuronCores).

Computes  -sum_i logsumexp_j( (xq @ Wq) @ (xk @ Wk)^T )[i, j]  for
Nq = Nk = 8192, D = 256 (fp32 inputs).

Strategy
--------
Algebra: scores = xq @ (Wq @ Wk^T) @ xk^T.  The host folds the two weight
matrices into A = Wq @ Wk^T (256x256, trivially cheap) so each core runs
two chained matmuls: QaT = A^T @ xqT (local rows), then score row-blocks
Qa @ xk^T, consumed tile-by-tile by an online logsumexp:

  per 128-row q-block, per 2048-wide j-tile:
      PE   : 8x  [128x128] @ [128x512] bf16 matmuls -> PSUM fp32
      DVE  : reduce_max (negated) over the PSUM tile -> -m_t
      ACT  : exp(s - m_t) with fused per-partition accumulate -> S_t

Each core ships per-tile (-m_t, S_t) pairs ([128, 8, 4] fp32 each) back to
the host, which merges them in float64:  lse = M + log(sum_t S_t e^{m_t-M}).
logsumexp compresses relative error, so bf16 matmul inputs are safe: tile
sums carry ~0.2% relative noise, which perturbs each row's lse by only
~log(1.002) ~ 2e-3 absolute against |lse| ~ 1.7e4.

Sharding: xq rows split across 8 cores (1024 rows each); A and xk^T are
replicated.  All inputs are pre-transposed + bf16-cast on the host so the
contraction dim (D=256, as 2 chunks of 128) lands on SBUF partitions.
"""

import sys

_TRN_REPO = "/opt/trn_rl_repo"
if _TRN_REPO not in sys.path:
    sys.path.insert(0, _TRN_REPO)

import ml_dtypes
import numpy as np

N_CORES = 8
NQ, NK, D = 8192, 8192, 256
ROWS = NQ // N_CORES        # q-rows per core
QB = ROWS // 128            # q-blocks (of 128 rows) per core
JT = 8                      # j-tiles per q-block row sweep
JW = NK // JT               # j-tile width (1024 fp32 = 2 PSUM banks)
BF16 = ml_dtypes.bfloat16

_COMPILED = None


def _build_nc():
    import concourse.bacc as bacc
    import concourse.mybir as mybir
    import concourse.tile as tile

    nc = bacc.Bacc("TRN2", target_bir_lowering=False, debug=False)
    f32 = mybir.dt.float32
    bf16 = mybir.dt.bfloat16
    EXP = mybir.ActivationFunctionType.Exp
    AX = mybir.AxisListType.X

    amat_d = nc.dram_tensor("amat", [2, 128, D], bf16, kind="ExternalInput").ap()
    xqt_d = nc.dram_tensor("xqt", [2, 128, ROWS], bf16, kind="ExternalInput").ap()
    xkt_d = nc.dram_tensor("xkt", [2, 128, NK], bf16, kind="ExternalInput").ap()
    negm_d = nc.dram_tensor("negm", [128, QB, JT], f32, kind="ExternalOutput").ap()
    ssum_d = nc.dram_tensor("ssum", [128, QB, JT], f32, kind="ExternalOutput").ap()

    with tile.TileContext(nc) as tc:
        with (
            tc.tile_pool(name="big", bufs=1) as big,
            tc.tile_pool(name="psum", bufs=4, space="PSUM") as psum,
        ):
            negm = big.tile([128, QB, JT], f32, name="negm", tag="negm")
            ssum = big.tile([128, QB, JT], f32, name="ssum", tag="ssum")
            amat = [big.tile([128, D], bf16, name=f"amat{e}", tag=f"amat{e}") for e in range(2)]
            xqt = [big.tile([128, ROWS], bf16, name=f"xqt{e}", tag=f"xqt{e}") for e in range(2)]
            xkt = [
                [big.tile([128, JW], bf16, name=f"xkt{e}_{j}", tag=f"xkt{e}_{j}") for j in range(JT)]
                for e in range(2)
            ]
            qat = [big.tile([128, ROWS], bf16, name=f"qat{d}", tag=f"qat{d}") for d in range(2)]

            # PE warm-up: dense dummy matmuls at t=0 so the HAM clock-gate
            # reaches 8/8 (2.4 GHz) before real matmuls start; otherwise the
            # whole kernel runs at the 1.2 GHz cold rate (measured 407ns/MM).
            scratch = big.tile([128, 512], bf16, name="warm_sb", tag="warm_sb")
            nc.gpsimd.memset(scratch[:], 0.0)
            warm_ps = psum.tile([128, JW], f32, name="warm_ps", tag="ps")
            for w in range(16):
                nc.tensor.matmul(
                    warm_ps[:, :512],
                    scratch[:, :128],
                    scratch[:],
                    start=True,
                    stop=True,
                )

            for e in range(2):
                nc.sync.dma_start(amat[e][:], amat_d[e])
                nc.sync.dma_start(xqt[e][:], xqt_d[e])
            for j in range(JT):
                for e in range(2):
                    nc.sync.dma_start(xkt[e][j][:], xkt_d[e][:, j * JW : (j + 1) * JW])

            # QaT[d-chunk] = sum_e (A[e-chunk, d-cols])^T @ xqT[e-chunk].
            # n-outer / per-(n,d) psum so the first half of qat (cols 0:512,
            # feeding q-blocks 0-3) is ready after two copies.
            for n in range(ROWS // 512):
                for d in range(2):
                    psq = psum.tile([128, 512], f32, name="psq", tag="ps")
                    for e in range(2):
                        nc.tensor.matmul(
                            psq[:],
                            amat[e][:, d * 128 : (d + 1) * 128],
                            xqt[e][:, n * 512 : (n + 1) * 512],
                            start=(e == 0),
                            stop=(e == 1),
                        )
                    nc.scalar.copy(qat[d][:, n * 512 : (n + 1) * 512], psq[:])

            # score tiles + online logsumexp partials.  d-outer so the
            # stationary operand is swapped twice per tile instead of per
            # matmul (per-MM LDWEIGHTS measured 418ns/MM vs 216 streaming).
            for b in range(QB):
                for j in range(JT):
                    ps = psum.tile([128, JW], f32, name="ps", tag="ps")
                    for d in range(2):
                        for s in range(JW // 512):
                            nc.tensor.matmul(
                                ps[:, s * 512 : (s + 1) * 512],
                                qat[d][:, b * 128 : (b + 1) * 128],
                                xkt[d][j][:, s * 512 : (s + 1) * 512],
                                start=(d == 0),
                                stop=(d == 1),
                            )
                    nc.vector.reduce_max(
                        negm[:, b, j : j + 1], ps[:], axis=AX, negate=True
                    )
                    nc.scalar.activation(
                        ps[:],
                        ps[:],
                        EXP,
                        bias=negm[:, b, j : j + 1],
                        scale=1.0,
                        accum_out=ssum[:, b, j : j + 1],
                    )

            nc.sync.dma_start(negm_d[:], negm[:])
            nc.sync.dma_start(ssum_d[:], ssum[:])

    nc.compile()
    return nc


def _get_compiled():
    global _COMPILED
    if _COMPILED is None:
        _COMPILED = _build_nc()
    return _COMPILED


def make_in_maps(xq, xk, Wq, Wk):
    """Host-side shard + layout prep: transpose, bf16-cast, fold A = Wq Wk^T."""
    A = (Wq.astype(np.float32) @ Wk.astype(np.float32).T).astype(BF16)
    amat = np.ascontiguousarray(A.reshape(2, 128, D))
    xkt = np.ascontiguousarray(xk.T.astype(BF16).reshape(2, 128, NK))
    in_maps = []
    for c in range(N_CORES):
        xq_c = xq[c * ROWS : (c + 1) * ROWS]
        xqt = np.ascontiguousarray(xq_c.T.astype(BF16).reshape(2, 128, ROWS))
        in_maps.append({"amat": amat, "xqt": xqt, "xkt": xkt})
    return in_maps


def combine_outputs(results):
    """Merge per-core (-m_t, S_t) tile stats into the final scalar (float64)."""
    total = 0.0
    for res in results:
        m = -res["negm"].astype(np.float64)  # [128, QB, JT] tile maxima
        s = res["ssum"].astype(np.float64)  # [128, QB, JT] tile exp-sums
        mrow = m.max(axis=-1)
        t = (s * np.exp(m - mrow[..., None])).sum(axis=-1)
        lse = mrow + np.log(t)
        total += lse.sum()
    return np.float32(-total)


def kernel(xq, xk, Wq, Wk):
    from concourse.bass_utils import run_bass_kernel_spmd

    xq = np.asarray(xq, dtype=np.float32)
    xk = np.asarray(xk, dtype=np.float32)
    Wq = np.asarray(Wq, dtype=np.float32)
    Wk = np.asarray(Wk, dtype=np.float32)

    nc = _get_compiled()
    in_maps = make_in_maps(xq, xk, Wq, Wk)
    out = run_bass_kernel_spmd(nc, in_maps, list(range(N_CORES)))
    return combine_outputs(out.results)


# revision 10
# speedup vs baseline: 1.3749x; 1.0201x over previous
"""Distributed cross-attention energy kernel for one TRN2 chip (8 NeuronCores).

Computes  -sum_i logsumexp_j( (xq @ Wq) @ (xk @ Wk)^T )[i, j]  for
Nq = Nk = 8192, D = 256 (fp32 inputs).

Strategy
--------
Algebra: scores = xq @ (Wq @ Wk^T) @ xk^T.  The host folds the two weight
matrices into A = Wq @ Wk^T (256x256, trivially cheap) so each core runs
two chained matmuls: QaT = A^T @ xqT (local rows), then score row-blocks
Qa @ xk^T, consumed tile-by-tile by an online logsumexp:

  per 128-row q-block, per 2048-wide j-tile:
      PE   : 8x  [128x128] @ [128x512] bf16 matmuls -> PSUM fp32
      DVE  : reduce_max (negated) over the PSUM tile -> -m_t
      ACT  : exp(s - m_t) with fused per-partition accumulate -> S_t

Each core ships per-tile (-m_t, S_t) pairs ([128, 8, 4] fp32 each) back to
the host, which merges them in float64:  lse = M + log(sum_t S_t e^{m_t-M}).
logsumexp compresses relative error, so bf16 matmul inputs are safe: tile
sums carry ~0.2% relative noise, which perturbs each row's lse by only
~log(1.002) ~ 2e-3 absolute against |lse| ~ 1.7e4.

Sharding: xq rows split across 8 cores (1024 rows each); A and xk^T are
replicated.  All inputs are pre-transposed + bf16-cast on the host so the
contraction dim (D=256, as 2 chunks of 128) lands on SBUF partitions.
"""

import sys

_TRN_REPO = "/opt/trn_rl_repo"
if _TRN_REPO not in sys.path:
    sys.path.insert(0, _TRN_REPO)

import ml_dtypes
import numpy as np

N_CORES = 8
NQ, NK, D = 8192, 8192, 256
ROWS = NQ // N_CORES        # q-rows per core
QB = ROWS // 128            # q-blocks (of 128 rows) per core
JT = 8                      # j-tiles per q-block row sweep
JW = NK // JT               # j-tile width (1024 fp32 = 2 PSUM banks)
BF16 = ml_dtypes.bfloat16

_COMPILED = None


def _build_nc():
    import concourse.bacc as bacc
    import concourse.mybir as mybir
    import concourse.tile as tile

    nc = bacc.Bacc("TRN2", target_bir_lowering=False, debug=False)
    f32 = mybir.dt.float32
    bf16 = mybir.dt.bfloat16
    EXP = mybir.ActivationFunctionType.Exp
    AX = mybir.AxisListType.X

    amat_d = nc.dram_tensor("amat", [2, 128, D], bf16, kind="ExternalInput").ap()
    xqt_d = nc.dram_tensor("xqt", [2, 128, ROWS], bf16, kind="ExternalInput").ap()
    xkt_d = nc.dram_tensor("xkt", [2, 128, NK], bf16, kind="ExternalInput").ap()
    negm_d = nc.dram_tensor("negm", [128, QB, JT], f32, kind="ExternalOutput").ap()
    ssum_d = nc.dram_tensor("ssum", [128, QB, JT], f32, kind="ExternalOutput").ap()

    with tile.TileContext(nc) as tc:
        with (
            tc.tile_pool(name="big", bufs=1) as big,
            tc.tile_pool(name="psum", bufs=4, space="PSUM") as psum,
        ):
            negm = big.tile([128, QB, JT], f32, name="negm", tag="negm")
            ssum = big.tile([128, QB, JT], f32, name="ssum", tag="ssum")
            amat = [big.tile([128, D], bf16, name=f"amat{e}", tag=f"amat{e}") for e in range(2)]
            xqt = [big.tile([128, ROWS], bf16, name=f"xqt{e}", tag=f"xqt{e}") for e in range(2)]
            xkt = [
                [big.tile([128, JW], bf16, name=f"xkt{e}_{j}", tag=f"xkt{e}_{j}") for j in range(JT)]
                for e in range(2)
            ]
            qat = [big.tile([128, ROWS], bf16, name=f"qat{d}", tag=f"qat{d}") for d in range(2)]

            # PE warm-up: dense dummy matmuls at t=0 so the HAM clock-gate
            # reaches 8/8 (2.4 GHz) before real matmuls start; otherwise the
            # whole kernel runs at the 1.2 GHz cold rate (measured 407ns/MM).
            scratch = big.tile([128, 512], bf16, name="warm_sb", tag="warm_sb")
            nc.gpsimd.memset(scratch[:], 0.0)
            warm_ps = psum.tile([128, JW], f32, name="warm_ps", tag="ps")
            for w in range(12):
                nc.tensor.matmul(
                    warm_ps[:, :512],
                    scratch[:, :128],
                    scratch[:],
                    start=True,
                    stop=True,
                )

            for e in range(2):
                nc.sync.dma_start(amat[e][:], amat_d[e])
                nc.sync.dma_start(xqt[e][:], xqt_d[e])
            for j in range(JT):
                for e in range(2):
                    nc.sync.dma_start(xkt[e][j][:], xkt_d[e][:, j * JW : (j + 1) * JW])

            # QaT[d-chunk] = sum_e (A[e-chunk, d-cols])^T @ xqT[e-chunk].
            # Emitted in two column-halves: half n feeds q-blocks 4n..4n+3,
            # so half 1 is deferred until after the first score tiles to
            # keep its Scalar-engine copies out of the first exp's FIFO path.
            def qat_half(n):
                for d in range(2):
                    psq = psum.tile([128, 512], f32, name="psq", tag="ps")
                    for e in range(2):
                        nc.tensor.matmul(
                            psq[:],
                            amat[e][:, d * 128 : (d + 1) * 128],
                            xqt[e][:, n * 512 : (n + 1) * 512],
                            start=(e == 0),
                            stop=(e == 1),
                        )
                    nc.scalar.copy(qat[d][:, n * 512 : (n + 1) * 512], psq[:])

            # score tile + online logsumexp partials.  d-outer so the
            # stationary operand is swapped twice per tile instead of per
            # matmul (per-MM LDWEIGHTS measured 418ns/MM vs 216 streaming).
            def score_tile(b, j):
                ps = psum.tile([128, JW], f32, name="ps", tag="ps")
                for d in range(2):
                    for s in range(JW // 512):
                        nc.tensor.matmul(
                            ps[:, s * 512 : (s + 1) * 512],
                            qat[d][:, b * 128 : (b + 1) * 128],
                            xkt[d][j][:, s * 512 : (s + 1) * 512],
                            start=(d == 0),
                            stop=(d == 1),
                        )
                nc.vector.reduce_max(
                    negm[:, b, j : j + 1], ps[:], axis=AX, negate=True
                )
                nc.scalar.activation(
                    ps[:],
                    ps[:],
                    EXP,
                    bias=negm[:, b, j : j + 1],
                    scale=1.0,
                    accum_out=ssum[:, b, j : j + 1],
                )

            qat_half(0)
            for j in range(JT):
                score_tile(0, j)
            qat_half(1)
            for b in range(1, QB):
                for j in range(JT):
                    score_tile(b, j)

            nc.sync.dma_start(negm_d[:], negm[:])
            nc.sync.dma_start(ssum_d[:], ssum[:])

    nc.compile()
    return nc


def _get_compiled():
    global _COMPILED
    if _COMPILED is None:
        _COMPILED = _build_nc()
    return _COMPILED


def make_in_maps(xq, xk, Wq, Wk):
    """Host-side shard + layout prep: transpose, bf16-cast, fold A = Wq Wk^T."""
    A = (Wq.astype(np.float32) @ Wk.astype(np.float32).T).astype(BF16)
    amat = np.ascontiguousarray(A.reshape(2, 128, D))
    xkt = np.ascontiguousarray(xk.T.astype(BF16).reshape(2, 128, NK))
    in_maps = []
    for c in range(N_CORES):
        xq_c = xq[c * ROWS : (c + 1) * ROWS]
        xqt = np.ascontiguousarray(xq_c.T.astype(BF16).reshape(2, 128, ROWS))
        in_maps.append({"amat": amat, "xqt": xqt, "xkt": xkt})
    return in_maps


def combine_outputs(results):
    """Merge per-core (-m_t, S_t) tile stats into the final scalar (float64)."""
    total = 0.0
    for res in results:
        m = -res["negm"].astype(np.float64)  # [128, QB, JT] tile maxima
        s = res["ssum"].astype(np.float64)  # [128, QB, JT] tile exp-sums
        mrow = m.max(axis=-1)
        t = (s * np.exp(m - mrow[..., None])).sum(axis=-1)
        lse = mrow + np.log(t)
        total += lse.sum()
    return np.float32(-total)


def kernel(xq, xk, Wq, Wk):
    from concourse.bass_utils import run_bass_kernel_spmd

    xq = np.asarray(xq, dtype=np.float32)
    xk = np.asarray(xk, dtype=np.float32)
    Wq = np.asarray(Wq, dtype=np.float32)
    Wk = np.asarray(Wk, dtype=np.float32)

    nc = _get_compiled()
    in_maps = make_in_maps(xq, xk, Wq, Wk)
    out = run_bass_kernel_spmd(nc, in_maps, list(range(N_CORES)))
    return combine_outputs(out.results)
